# revision 44
# baseline (speedup 1.0000x reference)
"""AttnBlock (GroupNorm + single-head 1x1-conv attention + residual) on 8
Trainium2 NeuronCores.

Sharding: data-parallel over batch (4) x sequence-parallel over query tokens
(2 halves of 4096). Each core receives its batch element with the spatial
columns rotated so that its 2048 query tokens are always columns 0:2047 —
attention is invariant to key order, so one shared NEFF serves all cores.

All heavy matmuls run in fp8e4 with MatmulPerfMode.DoubleRow (2 contraction
chunks of 128 per instruction = 2x PE throughput). Attention uses the
transposed dataflow S^T[key, query]: exp is applied in [k, q] layout so no
PE transposes are needed, the softmax denominator comes from a ones-
stationary matmul, and the output projection is pre-fused into the v
projection on the host (wvo = wo @ wv), eliminating phase 4 entirely.

Scaling: weights are scaled x32 on the host to keep fp8 values in the
normal range; the 1/1024 (q,k) descale is folded into the exp scale and
the 1/32 (v') descale into the output epilogue.
"""

import numpy as np

P = 128
C = 512
KC = C // P          # 4 channel chunks of 128
N = 4096             # tokens (64*64)
NH = N // 2          # query tokens per core
G = 32               # groupnorm groups
GS = C // G          # 16 channels per group
EPS = 1e-6
N_CORES = 8
QW = 512             # query chunk width (PSUM bank = 512 fp32 caps matmul out)
QC = NH // QW        # 4 query chunks
KT = N // P          # 32 key chunks of 128
SCALE = float(C) ** -0.5
WSC = 32.0           # host-side weight scale
LN8 = 2.0794415416798357

_CACHE = {}


def _apply_walrus_workarounds():
    """The walrus build in this container rejects any instruction carrying
    more than one semaphore wait ("Too many sync wait commands"). Split extra
    waits onto same-engine single-wait NOPs committed just before, and split
    the final TileContext drain the same way."""
    import concourse.tile as tile
    from concourse import mybir

    if getattr(tile.TileContext, "_walrus_wait_split", False):
        return

    _orig_commit = tile.TileContext._commit_instruction

    def _split_waits_commit(self, inst, lazy_reg_writes=True):
        si = inst.sync_info
        if si is not None and si.on_wait and len(si.on_wait) > 1 \
                and inst.engine != mybir.EngineType.Unassigned:
            waits = list(si.on_wait)
            si.on_wait = waits[-1:]
            for w in waits[:-1]:
                nop = mybir.InstNoOp(
                    name=self.nc.get_next_instruction_name(),
                    engine=inst.engine,
                    sync_info=mybir.SyncInfo(on_wait=[w], on_update=[]),
                    bass_nofuse=True,
                )
                _orig_commit(self, nop, lazy_reg_writes=False)
        return _orig_commit(self, inst, lazy_reg_writes=lazy_reg_writes)

    def _split_drain_and_barrier(self, tick_clock, wait_clock):
        nc = self.nc
        drain_inst = nc.sync.drain()
        wait_clock.add_sem_waits(
            drain_inst.ins, tile.ScopedClock({None: tick_clock.global_clock})
        )
        si = drain_inst.ins.sync_info
        waits = list(si.on_wait) if si is not None else []
        if len(waits) > 1:
            si.on_wait = waits[:1]
            for w in waits[1:]:
                d2 = nc.sync.drain()
                d2.ins.sync_info = mybir.SyncInfo(on_wait=[w], on_update=[])

        import os
        nc.all_engine_barrier()
        assert self.sems is not None
        popped = nc._tile_sem_poison_stack.pop()
        assert popped is self._sem_poison
        if os.environ.get("KERNEL_SKIP_SEM_RESET") != "1":
            nc.clear_and_free_semaphores(list(self.sems.allocated().values()))
            nc.all_engine_barrier()

    tile.TileContext._commit_instruction = _split_waits_commit
    tile.TileContext._drain_and_barrier = _split_drain_and_barrier
    tile.TileContext._walrus_wait_split = True


def _build():
    """Trace the Bass/Tile program once; returns the Bass module."""
    import concourse.bass as bass
    import concourse.tile as tile
    from concourse import mybir

    _apply_walrus_workarounds()

    DR = mybir.MatmulPerfMode.DoubleRow
    DT8 = mybir.dt.float8e4
    DT = mybir.dt.float16
    F32 = mybir.dt.float32
    AT = mybir.AluOpType

    nc = bass.Bass("TRN2", target_bir_lowering=False, debug=False, num_devices=1)

    xr = nc.dram_tensor("xr", [C, N], DT, kind="ExternalInput").ap()
    wq8 = nc.dram_tensor("wq8", [C, C], DT8, kind="ExternalInput").ap()
    wk8 = nc.dram_tensor("wk8", [C, C], DT8, kind="ExternalInput").ap()
    wvo8 = nc.dram_tensor("wvo8", [C, C], DT8, kind="ExternalInput").ap()
    # packed per-channel vectors: [32*bq, wo@bv+bo, gamma, beta]
    bvec = nc.dram_tensor("bvec", [4, C], F32, kind="ExternalInput").ap()
    gavg = nc.dram_tensor("gavg", [P, P], F32, kind="ExternalInput").ap()
    ident = nc.dram_tensor("ident", [P, P], DT, kind="ExternalInput").ap()
    y = nc.dram_tensor("y", [C, NH], DT, kind="ExternalOutput").ap()

    xr_t = xr.rearrange("(kc p) n -> kc p n", p=P)     # [4, 128, 4096]
    y_t = y.rearrange("(oc p) n -> oc p n", p=P)       # [4, 128, 2048]

    with tile.TileContext(nc) as tc:
        import contextlib
        ctx = contextlib.ExitStack()
        with ctx:
            consts = ctx.enter_context(tc.tile_pool(name="consts", bufs=1))
            big = ctx.enter_context(tc.tile_pool(name="big", bufs=1))
            scp = ctx.enter_context(tc.tile_pool(name="scp", bufs=2))
            small = ctx.enter_context(tc.tile_pool(name="small", bufs=4))
            rp = ctx.enter_context(tc.tile_pool(name="rp", bufs=2))
            e2p = ctx.enter_context(tc.tile_pool(name="e2p", bufs=2))
            ps = ctx.enter_context(tc.tile_pool(name="ps", bufs=4, space="PSUM"))

            # ---- phase 1: GroupNorm -> hn (fp8) --------------------------
            # Per-chunk sums via hierarchical fp16 adds + reduce on DVE,
            # sum-of-squares via ACT Square with fused accumulator; the
            # normalizes are spread across Pool/DVE/ACT so no single engine
            # serializes the head.
            hn = big.tile([P, KC, N], DT8, tag="hn")
            x_full = big.tile([P, KC, N], DT, tag="xf")
            bv_sb = None
            for kc in range(KC):
                x_c = x_full[:, kc, :]
                nc.sync.dma_start(x_c[:], xr_t[kc])
                if bv_sb is None:
                    bv_sb = consts.tile([P, 4, KC], F32, tag="bvec")
                    nc.gpsimd.dma_start(
                        bv_sb[:], bvec.rearrange("v (kc p) -> p v kc", p=P))
                    b_sb = {n: bv_sb[:, vi, :] for vi, n in
                            enumerate(("bq", "bc", "gam", "bet"))}
                    gavg_sb = consts.tile([P, P], F32, tag="gavg")
                    nc.gpsimd.dma_start(gavg_sb[:], gavg)
                    ident_sb = consts.tile([P, P], DT, tag="ident")
                    nc.gpsimd.dma_start(ident_sb[:], ident)
                    ones8_sb = consts.tile([P, 2, P], DT8, tag="ones8")
                    nc.vector.memset(ones8_sb[:], 1.0)
                    eps_sb = consts.tile([P, 1], F32, tag="eps")
                    nc.vector.memset(eps_sb[:], EPS)
                    ebias = consts.tile([P, 1], F32, tag="ebias")
                    nc.vector.memset(ebias[:], -LN8)
                    # warm up the cold gpsimd DSP: its first ops run 3-6x
                    # slow, which otherwise lands on the GN critical path
                    pwu = consts.tile([P, 1], F32, tag="pwu")
                    for _ in range(6):
                        nc.gpsimd.tensor_tensor(
                            pwu[:], eps_sb[:], eps_sb[:], AT.add)


            # weights on the sync queue, behind the x chunks: no competition
            # with x for bandwidth early, no gpsimd-engine queueing later
            w_sb = {}
            for wname, wap in (("wq", wq8), ("wk", wk8), ("wvo", wvo8)):
                t = consts.tile([P, KC, C], DT8, tag=f"w_{wname}")
                nc.sync.dma_start(
                    t[:], wap.rearrange("(kc p) o -> p kc o", p=P))
                w_sb[wname] = t

            stats = []
            g_all = small.tile([P, 2, KC], F32, tag="gall")
            for kc in range(KC):
                x_c = x_full[:, kc, :]
                mv2 = small.tile([P, 2], F32, tag="mv2", name=f"mv2_{kc}")
                # hierarchical sum: fp16 halves+quarters (DVE 2x path, short
                # bursts so parked small ops resume quickly) + reduce
                sc = scp.tile([P, N // 2], DT, tag="sc")
                nc.vector.tensor_tensor(
                    sc[:], x_c[:, :N // 2], x_c[:, N // 2:], AT.add)
                sc4 = scp.tile([P, N // 4], DT, tag="sc4")
                nc.vector.tensor_tensor(
                    sc4[:], sc[:, :N // 4], sc[:, N // 4:], AT.add)
                nc.vector.tensor_reduce(
                    mv2[:, 0:1], sc4[:], mybir.AxisListType.X, AT.add)
                # hn[:, kc] is throwaway scratch here, overwritten below
                nc.scalar.activation(
                    hn[:, kc, 0:N], x_c[:], mybir.ActivationFunctionType.Square,
                    accum_out=mv2[:, 1:2])
                g_ps = ps.tile([P, 2], F32, tag="pair", name=f"gn{kc}")
                nc.tensor.matmul(g_ps[:], gavg_sb[:], mv2[:], start=True, stop=True)
                nc.vector.tensor_copy(g_all[:, :, kc], g_ps[:])

                if kc % 2 == 0:
                    continue
                # batched chain for the (kc-1, kc) pair: [P,2]-wide ops,
                # half the op count and no cross-pair engine entanglement
                p0 = kc - 1
                mean2 = g_all[:, 0, p0:p0 + 2]
                e22 = g_all[:, 1, p0:p0 + 2]
                ce = nc.gpsimd if kc == 1 else nc.vector
                var2 = small.tile([P, 2], F32, tag="var", name=f"var{kc}")
                ce.tensor_tensor(var2[:], mean2, mean2, AT.mult)
                ce.tensor_tensor(var2[:], e22, var2[:], AT.subtract)
                sq2 = small.tile([P, 2], F32, tag="sq", name=f"sq{kc}")
                nc.scalar.activation(
                    sq2[:], var2[:], mybir.ActivationFunctionType.Sqrt,
                    bias=eps_sb[:], scale=1.0)
                rstd2 = small.tile([P, 2], F32, tag="rstd", name=f"rstd{kc}")
                nc.vector.reciprocal(rstd2[:], sq2[:])
                scl2 = small.tile([P, 2], F32, tag="scl", name=f"scl{kc}")
                ce.tensor_tensor(
                    scl2[:], rstd2[:], b_sb["gam"][:, p0:p0 + 2], AT.mult)
                sh2 = small.tile([P, 2], F32, tag="sh", name=f"sh{kc}")
                ce.tensor_tensor(sh2[:], mean2, scl2[:], AT.mult)
                ce.tensor_tensor(
                    sh2[:], b_sb["bet"][:, p0:p0 + 2], sh2[:], AT.subtract)

                # normalize both chunks of the pair right away
                for h in range(2):
                    c2 = p0 + h
                    scl = scl2[:, h:h + 1]
                    sh = sh2[:, h:h + 1]
                    x_h = x_full[:, c2, :]
                    if c2 == 0:
                        nc.gpsimd.tensor_scalar(
                            out=hn[:, c2, :], in0=x_h[:],
                            scalar1=scl, scalar2=sh,
                            op0=AT.mult, op1=AT.add)
                    elif c2 == 1:
                        nc.vector.tensor_scalar(
                            out=hn[:, c2, :], in0=x_h[:],
                            scalar1=scl, scalar2=sh,
                            op0=AT.mult, op1=AT.add)
                    elif c2 == 2:
                        nc.scalar.activation(
                            hn[:, c2, :], x_h[:],
                            mybir.ActivationFunctionType.Identity,
                            bias=sh, scale=scl)
                    else:
                        nc.gpsimd.tensor_scalar(
                            out=hn[:, c2, :N // 2], in0=x_h[:, :N // 2],
                            scalar1=scl, scalar2=sh,
                            op0=AT.mult, op1=AT.add)
                        nc.vector.tensor_scalar(
                            out=hn[:, c2, N // 2:], in0=x_h[:, N // 2:],
                            scalar1=scl, scalar2=sh,
                            op0=AT.mult, op1=AT.add)

            # HAM warm-up: keep the PE busy through the phase-1 tail.
            warm_ps = ps.tile([P, 512], F32, tag="pair", name="warm")
            for wi in range(8):
                nc.tensor.matmul(warm_ps[:], ident_sb[:], x_full[:, 2, :512],
                                 start=(wi == 0), stop=(wi == 7))

            # ---- phase 2: projections (all DR fp8) ------------------------
            k_sb = big.tile([P, KC, N], DT8, tag="k")
            q_sb = big.tile([P, KC, NH], DT8, tag="q")
            v_sb = big.tile([P, KT, C], DT8, tag="v")

            def stage_kq(w, dst_sb, tsp, bias, eng):
                """projection of token chunk tsp (1024 tokens) -> dst."""
                for ocp in range(2):
                    pp = ps.tile([P, 2, QW], F32, tag="pair")
                    for h in range(2):
                        oc = 2 * ocp + h
                        for j in range(2):
                            nc.tensor.matmul(
                                pp[:, h, :],
                                w[:, 2 * j:2 * j + 2, oc * P:(oc + 1) * P],
                                hn[:, 2 * j:2 * j + 2,
                                   tsp * QW:(tsp + 1) * QW],
                                start=(j == 0), stop=(j == 1), perf_mode=DR)
                    for h in range(2):
                        oc = 2 * ocp + h
                        dst = dst_sb[:, oc, tsp * QW:(tsp + 1) * QW]
                        e = eng(oc)
                        if bias is None:
                            if e == "act":
                                nc.scalar.copy(dst, pp[:, h, :])
                            else:
                                nc.vector.tensor_copy(dst, pp[:, h, :])
                        else:
                            if e == "act":
                                nc.scalar.activation(
                                    dst, pp[:, h, :],
                                    mybir.ActivationFunctionType.Identity,
                                    bias=bias[:, oc:oc + 1], scale=1.0)
                            else:
                                nc.vector.tensor_scalar(
                                    out=dst, in0=pp[:, h, :],
                                    scalar1=bias[:, oc:oc + 1], scalar2=None,
                                    op0=AT.add)

            def stage_v_pair(u):
                """v' projection for token chunks 2u, 2u+1 (128 tokens each)."""
                pp = ps.tile([P, 2, 512], F32, tag="pair")
                for h in range(2):
                    jc = 2 * u + h
                    for j in range(2):
                        nc.tensor.matmul(
                            pp[:, h, :],
                            hn[:, 2 * j:2 * j + 2, jc * P:(jc + 1) * P],
                            w_sb["wvo"][:, 2 * j:2 * j + 2, :],
                            start=(j == 0), stop=(j == 1), perf_mode=DR)
                for h in range(2):
                    jc = 2 * u + h
                    if jc % 2 == 0:
                        nc.vector.tensor_copy(v_sb[:, jc, :], pp[:, h, :])
                    else:
                        nc.scalar.copy(v_sb[:, jc, :], pp[:, h, :])

            # q chunk 0 first (unlocks attention), then all of k.
            stage_kq(w_sb["wq"], q_sb, 0, b_sb["bq"], lambda oc: "act")
            for tsp in range(N // QW):
                stage_kq(w_sb["wk"], k_sb, tsp, None,
                         lambda oc: "dve" if oc % 2 == 0 else "act")

            # ---- phase 3: attention ---------------------------------------
            def b1(qc, extra):
                e2 = e2p.tile([P, KT, QW], DT8, tag="e2", name=f"e2_{qc}")
                for u in range(KT // 2):
                    s2 = ps.tile([P, 2, QW], F32, tag="pair")
                    for h in range(2):
                        kt = 2 * u + h
                        for j in range(2):
                            nc.tensor.matmul(
                                s2[:, h, :],
                                k_sb[:, 2 * j:2 * j + 2, kt * P:(kt + 1) * P],
                                q_sb[:, 2 * j:2 * j + 2,
                                     qc * QW:(qc + 1) * QW],
                                start=(j == 0), stop=(j == 1), perf_mode=DR)
                    nc.scalar.activation(
                        e2[:, 2 * u:2 * u + 2, :], s2[:],
                        mybir.ActivationFunctionType.Exp,
                        bias=ebias[:], scale=SCALE / (WSC * WSC))
                    if extra is not None:
                        extra(u)
                return e2

            def b2(qc, e2):
                d_ps = ps.tile([P, 2, QW], F32, tag="pair", name=f"d{qc}")
                for u in range(KT // 2):
                    nc.tensor.matmul(
                        d_ps[:, 0, :], ones8_sb[:], e2[:, 2 * u:2 * u + 2, :],
                        start=(u == 0), stop=(u == KT // 2 - 1), perf_mode=DR)
                recip = rp.tile([P, QW], F32, tag="recip")
                nc.vector.reciprocal(recip[:], d_ps[:, 0, :])

                for ccp in range(2):
                    o2 = ps.tile([P, 2, QW], F32, tag="pair",
                                 name=f"o{ccp}_{qc}")
                    for u in range(KT // 2):
                        for h in range(2):
                            cc = 2 * ccp + h
                            nc.tensor.matmul(
                                o2[:, h, :],
                                v_sb[:, 2 * u:2 * u + 2, cc * P:(cc + 1) * P],
                                e2[:, 2 * u:2 * u + 2, :],
                                start=(u == 0), stop=(u == KT // 2 - 1),
                                perf_mode=DR)
                    for h in range(2):
                        cc = 2 * ccp + h
                        r32 = rp.tile([P, QW], F32, tag="r32")
                        nc.vector.tensor_tensor(
                            r32[:], o2[:, h, :], recip[:], AT.mult)
                        nc.vector.tensor_scalar(
                            out=r32[:], in0=r32[:], scalar1=1.0 / WSC,
                            scalar2=b_sb["bc"][:, cc:cc + 1],
                            op0=AT.mult, op1=AT.add)
                        y16 = rp.tile([P, QW], DT, tag="y16")
                        nc.vector.tensor_tensor(
                            y16[:], r32[:],
                            x_full[:, cc, qc * QW:(qc + 1) * QW], AT.add)
                        nc.sync.dma_start(
                            y_t[cc][:, qc * QW:(qc + 1) * QW], y16[:])

            def extra_proj(u):
                # v' projection + q chunks 1..3 fill b1(0)'s exp-paced PE
                stage_v_pair(u)
                if u % 4 == 3 and u // 4 + 1 < QC:
                    stage_kq(w_sb["wq"], q_sb, u // 4 + 1, b_sb["bq"],
                             lambda oc: "dve" if oc % 2 == 0 else "act")

            e2 = b1(0, extra_proj)
            for qc in range(QC):
                b2(qc, e2)
                if qc + 1 < QC:
                    e2 = b1(qc + 1, None)

    return nc


def _prep_in_maps(inputs):
    import ml_dtypes
    F8 = ml_dtypes.float8_e4m3

    x = np.asarray(inputs["x"], np.float32).reshape(4, C, N)
    wq = np.asarray(inputs["wq"], np.float32)
    wk = np.asarray(inputs["wk"], np.float32)
    wv = np.asarray(inputs["wv"], np.float32)
    wo = np.asarray(inputs["wo"], np.float32)
    wvo = wo @ wv
    bvec = np.stack([
        np.asarray(inputs["bq"], np.float32) * WSC,
        wo @ np.asarray(inputs["bv"], np.float32)
        + np.asarray(inputs["bo"], np.float32),
        np.asarray(inputs["gamma"], np.float32),
        np.asarray(inputs["beta"], np.float32),
    ]).astype(np.float32)
    shared = {
        "wq8": np.ascontiguousarray(wq.T * WSC).astype(F8),
        "wk8": np.ascontiguousarray(wk.T * WSC).astype(F8),
        "wvo8": np.ascontiguousarray(wvo.T * WSC).astype(F8),
        "bvec": bvec,
        "gavg": (np.kron(np.eye(P // GS, dtype=np.float32),
                         np.ones((GS, GS), np.float32)) / (GS * N)),
        "ident": np.eye(P, dtype=np.float16),
    }
    in_maps = []
    for core in range(N_CORES):
        b, half = divmod(core, 2)
        xb = x[b]
        if half == 1:
            xrot = np.ascontiguousarray(
                np.concatenate([xb[:, NH:], xb[:, :NH]], axis=1))
        else:
            xrot = np.ascontiguousarray(xb)
        in_maps.append({"xr": xrot.astype(np.float16), **shared})
    return in_maps


def kernel_run(inputs, trace=False, trace_cores=None):
    """Run on all 8 cores; returns (full_output, BassKernelResults)."""
    from concourse.bass_utils import run_bass_kernel_spmd

    if "nc" not in _CACHE:
        _CACHE["nc"] = _build()
    nc = _CACHE["nc"]
    in_maps = _prep_in_maps(inputs)
    res = run_bass_kernel_spmd(
        nc, in_maps, core_ids=list(range(N_CORES)), trace=trace,
        trace_cores=trace_cores)
    out = np.empty((4, C, N), np.float32)
    for core in range(N_CORES):
        b, half = divmod(core, 2)
        out[b][:, half * NH:(half + 1) * NH] = res.results[core]["y"]
    return out.reshape(4, C, 64, 64), res


def kernel(**inputs):
    out, _ = kernel_run(inputs, trace=False)
    return out
